# revision 2
# baseline (speedup 1.0000x reference)
"""CoxLoss Trainium2 kernel (v7): two-level histogram/CDF, 8-way SPMD.

Structure (per core, MY=2048 own rows):
  k = s * 2^14 (f32 exact), a = i32(rne(s*128 - 0.5)) ~ floor(k/128)  [ACT]
  u = bf16(k - 128 a) in [0,128]                                      [DVE STT]
  Msuf[b,t] = sum_j w_j [k_j >= 128 b][u_j >= t]     (PE, 16 j-chunks)
  G2 = 2nd-order diff of Msuf^T + suffix fold  ->  AllReduce (64KB)
  risk_i = sum_b [k_i >= 128 b] * sum_t G2tot[t,b] [u_i >= t]
  loss_partial = (1/N) sum_i cen_i (ln risk_i - ln w_i); host sums cores.

Scheduling: the i-side staircases (dct/dab) are emitted AFTER the
collective so the in-order DVE queue executes them inside the CC wait
window; the G2 chain (which gates the CC trigger) runs right after the
j-side. ACT does the f32->i32 floor converts on both sides (consistent
rounding), and the DVE STT reads the i32 tiles directly.
"""
import numpy as np
import concourse.bass as bass
import concourse.mybir as mybir
from concourse.tile import TileContext
from concourse.bass_utils import run_bass_kernel_spmd

F32 = mybir.dt.float32
F32R = mybir.dt.float32r
I32 = mybir.dt.int32
BF16 = mybir.dt.bfloat16
AF = mybir.ActivationFunctionType
ALU = mybir.AluOpType

N = 16384
P = 128
NCORES = 8
MY = N // NCORES
JCH = MY // P             # 16
B1 = 128
B2 = 128
SCALE = float(B1 * B2)    # 2^14
QF = 512
NQ = MY // QF             # 4

C_S = 0
C_TH = JCH
C_CEN = 2 * JCH
C_PCOL = 3 * JCH
C_P128 = C_PCOL + 1
C_ONES = C_P128 + 1
C_NHALF = C_ONES + 1
C_IOB = C_NHALF + 1       # 128*b thresholds, B1 cols (replicated rows)
C_IOT = C_IOB + B1        # t thresholds, B2 cols
C_ID = C_IOT + B2
PACKW = C_ID + P


def legalize_waits(nc, max_waits=1):
    fn = nc.m.functions[0]
    for blk in fn.blocks:
        insts = blk.instructions
        out_list = []
        changed = False
        for ins in insts:
            si = ins.sync_info
            if si is not None and len(si.on_wait) > max_waits:
                waits = list(si.on_wait)
                keep = waits[:max_waits]
                for k, w in enumerate(waits[max_waits:]):
                    d = mybir.InstDrain(name=f"{ins.name}-w{k}", ins=[], outs=[])
                    d.engine = ins.engine
                    d.sync_info = mybir.SyncInfo(on_wait=[w], on_update=[])
                    out_list.append(d)
                si.on_wait = keep
                ins.sync_info = si
                changed = True
            out_list.append(ins)
        if changed:
            blk.instructions = out_list


def build(no_ar=False, early_cc=False):
    nc = bass.Bass()
    in_pack = nc.dram_tensor("in_pack", [P, PACKW], F32, kind="ExternalInput")
    in_row = nc.dram_tensor("in_row", [1, MY], F32, kind="ExternalInput")
    out = nc.dram_tensor("partial", [1, 1], F32, kind="ExternalOutput")

    g2_dram = nc.dram_tensor("g2_dram", [B2, B1], F32)
    g2_sh = nc.dram_tensor("g2_sh", [B2, B1], F32, addr_space="Shared")

    with TileContext(nc) as tc:
        with (
            tc.tile_pool(name="sb", bufs=1) as sb,
            tc.tile_pool(name="pms", bufs=1, space="PSUM") as pms,
            tc.tile_pool(name="ptr", bufs=1, space="PSUM") as ptr,
            tc.tile_pool(name="pr1", bufs=1, space="PSUM") as pr1,
            tc.tile_pool(name="pfin", bufs=1, space="PSUM") as pfin,
        ):
            # ---------------- input DMAs (sync + scalar queues)
            pack = sb.tile([P, PACKW], F32)
            nc.gpsimd.dma_start(out=pack, in_=in_pack[:, :])
            s_rep = sb.tile([P, MY], F32)
            nc.sync.dma_start(out=s_rep[:, 0:MY // 2],
                              in_=in_row[:, 0:MY // 2].to_broadcast([P, MY // 2]))
            nc.sync.dma_start(
                out=s_rep[:, MY // 2:MY],
                in_=in_row[:, MY // 2:MY].to_broadcast([P, MY // 2]))

            s_cols = pack[:, C_S:C_S + JCH]
            th_cols = pack[:, C_TH:C_TH + JCH]
            cen_cols = pack[:, C_CEN:C_CEN + JCH]
            pcol = pack[:, C_PCOL:C_PCOL + 1]
            p128col = pack[:, C_P128:C_P128 + 1]
            ones_col = pack[:, C_ONES:C_ONES + 1]
            nhalf_col = pack[:, C_NHALF:C_NHALF + 1]
            ident = pack[:, C_ID:C_ID + P]
            iotaB_rep = pack[:, C_IOB:C_IOB + B1]      # 128*b
            iotaT_rep = pack[:, C_IOT:C_IOT + B2]      # t

            # ---------------- ACT: sigmoid, i32 floors, k_rep
            w_colb = sb.tile([P, JCH], BF16)
            nc.scalar.activation(out=w_colb, in_=th_cols, func=AF.Sigmoid)
            # a = i32(rne(s*128 - 0.5)) ~ floor(s*128); same ACT convert on
            # both j and i sides keeps per-element keys consistent.
            ai_j = sb.tile([P, JCH], I32)
            nc.scalar.activation(out=ai_j, in_=s_cols, func=AF.Identity,
                                 scale=128.0, bias=nhalf_col)
            ai_i = sb.tile([P, MY], I32)
            nc.scalar.activation(out=ai_i, in_=s_rep, func=AF.Identity,
                                 scale=128.0, bias=nhalf_col)
            k_rep = sb.tile([P, MY], F32)
            nc.scalar.activation(out=k_rep, in_=s_rep, func=AF.Identity,
                                 scale=SCALE)

            # ---------------- DVE: j-side (gates the CC trigger)
            k_col = sb.tile([P, JCH], F32)
            nc.vector.tensor_scalar(out=k_col, in0=s_cols, scalar1=SCALE,
                                    scalar2=None, op0=ALU.mult)
            da_all = sb.tile([P, JCH * B1], BF16)
            da3 = da_all[:, :].rearrange("p (j b) -> p j b", j=JCH, b=B1)
            nc.vector.tensor_tensor(
                out=da3, in0=k_col[:, :, None].to_broadcast([P, JCH, B1]),
                in1=iotaB_rep[:, None, :].to_broadcast([P, JCH, B1]),
                op=ALU.is_ge)
            u_colb = sb.tile([P, JCH], BF16)
            nc.vector.scalar_tensor_tensor(out=u_colb, in0=ai_j, scalar=-128.0,
                                           in1=k_col, op0=ALU.mult, op1=ALU.add)
            dc_all = sb.tile([P, JCH * B2], BF16)
            dc3 = dc_all[:, :].rearrange("p (j t) -> p j t", j=JCH, t=B2)
            nc.vector.tensor_tensor(
                out=dc3, in0=u_colb[:, :, None].to_broadcast([P, JCH, B2]),
                in1=iotaT_rep[:, None, :].to_broadcast([P, JCH, B2]),
                op=ALU.is_ge)
            dcw_all = sb.tile([P, JCH * B2], BF16)
            dcw3 = dcw_all[:, :].rearrange("p (j t) -> p j t", j=JCH, t=B2)
            nc.vector.tensor_tensor(
                out=dcw3, in0=dc3,
                in1=w_colb[:, :, None].to_broadcast([P, JCH, B2]),
                op=ALU.mult)

            # ---------------- PE: Msuf accumulation
            msuf_ps = pms.tile([P, B2], F32, tag="ms")
            for jc in range(JCH):
                nc.tensor.matmul(msuf_ps[:, :],
                                 da_all[:, jc * B1:(jc + 1) * B1],
                                 dcw_all[:, jc * B2:(jc + 1) * B2],
                                 start=(jc == 0), stop=(jc == JCH - 1))

            # ---------------- G2 pipeline (pre-AR; linear in Msuf)
            msuf_sb = sb.tile([P, B2], F32)
            nc.scalar.copy(msuf_sb, msuf_ps[:, :])
            y_sb = sb.tile([P, B2], F32)
            nc.vector.tensor_copy(y_sb[:, 0:1], msuf_sb[:, 0:1])
            nc.vector.tensor_tensor(out=y_sb[:, 1:B2], in0=msuf_sb[:, 1:B2],
                                    in1=msuf_sb[:, 0:B2 - 1], op=ALU.subtract)
            tp = ptr.tile([P, P], F32, tag="tp")
            nc.tensor.matmul(tp[:, :], y_sb, ident, is_transpose=True)
            yt = sb.tile([P, B1 + 2], F32)
            nc.vector.memset(yt[:, 0:1], 0.0)
            nc.vector.memset(yt[:, B1 + 1:B1 + 2], 0.0)
            nc.scalar.copy(yt[:, 1:B1 + 1], tp[:, :])
            nc.scalar.copy(yt[:, 0:1], yt[:, 1:2])
            g2 = sb.tile([P, B1], F32)
            nc.vector.scalar_tensor_tensor(out=g2, in0=yt[:, 1:B1 + 1],
                                           scalar=2.0, in1=yt[:, 2:B1 + 2],
                                           op0=ALU.mult, op1=ALU.subtract)
            nc.vector.tensor_tensor(out=g2, in0=g2, in1=yt[:, 0:B1],
                                    op=ALU.subtract)
            hh = sb.tile([1, B1], F32)
            nc.vector.tensor_tensor(out=hh[:1, :], in0=yt[0:1, 2:B1 + 2],
                                    in1=yt[0:1, 1:B1 + 1], op=ALU.subtract)
            nc.vector.tensor_tensor(out=g2[0:1, :], in0=g2[0:1, :],
                                    in1=hh[:1, :], op=ALU.add)
            nc.vector.tensor_tensor(out=g2[0:1, 0:1], in0=g2[0:1, 0:1],
                                    in1=yt[0:1, 1:2], op=ALU.add)

            # ---------------- AllReduce of G2
            nc.gpsimd.dma_start(out=g2_dram[:, :], in_=g2)
            if no_ar:
                nc.gpsimd.dma_start(out=g2_sh[:, :], in_=g2_dram[:, :])
            else:
                nc.gpsimd.collective_compute(
                    "AllReduce", ALU.add,
                    ins=[g2_dram[:, :]], outs=[g2_sh[:, :]],
                    replica_groups=[list(range(NCORES))])
            m_ar = sb.tile([P, B1], F32)
            nc.gpsimd.dma_start(out=m_ar, in_=g2_sh[:, :])

            # ---------------- pre-AR epilogue pieces (run in the CC window)
            lnw = sb.tile([P, JCH], F32)
            nc.scalar.activation(out=lnw, in_=w_colb, func=AF.Ln)

            # ---------------- i-side staircases: scheduled into the CC wait
            # window (wait-until keeps them from preempting the G2 chain).
            u_repb = sb.tile([P, MY], BF16)
            dct = sb.tile([P, MY], BF16)
            dab = sb.tile([P, MY], BF16)
            with tc.tile_wait_until(0.030):
                nc.vector.scalar_tensor_tensor(out=u_repb, in0=ai_i,
                                               scalar=-128.0, in1=k_rep,
                                               op0=ALU.mult, op1=ALU.add)
                nc.vector.tensor_scalar(out=dct, in0=u_repb, scalar1=pcol,
                                        scalar2=None, op0=ALU.is_ge)
                nc.vector.tensor_scalar(out=dab, in0=k_rep, scalar1=p128col,
                                        scalar2=None, op0=ALU.is_ge)
            acc2s = sb.tile([P, 1], F32)
            junk2 = sb.tile([P, JCH], F32)
            acc2 = sb.tile([P, 1], F32)
            nc.vector.scalar_tensor_tensor(out=junk2, in0=lnw, scalar=1.0,
                                           in1=cen_cols, op0=ALU.mult,
                                           op1=ALU.mult, accum_out=acc2)
            nc.vector.tensor_scalar(out=acc2s, in0=acc2, scalar1=1.0 / N,
                                    scalar2=None, op0=ALU.mult)

            # ---------------- post-AR: hi/lo split, R1, prod, reduce
            g2hi = sb.tile([P, B1], BF16)
            nc.vector.tensor_copy(g2hi, m_ar)
            g2lo = sb.tile([P, B1], BF16)
            nc.vector.tensor_tensor(out=g2lo, in0=m_ar, in1=g2hi,
                                    op=ALU.subtract)

            risk_pm = pfin.tile([P, JCH], F32, tag="riskpm")
            r1s, prods = [], []
            for it in range(NQ):
                r1 = pr1.tile([P, QF], F32, tag=f"r1_{it}", name=f"r1_{it}")
                nc.tensor.matmul(r1[:, :], g2hi,
                                 dct[:, it * QF:(it + 1) * QF],
                                 start=True, stop=False)
                nc.tensor.matmul(r1[:, :], g2lo,
                                 dct[:, it * QF:(it + 1) * QF],
                                 start=False, stop=True)
                r1s.append(r1)
            for it in range(NQ):
                prod = sb.tile([P, QF], F32, tag=f"prod{it}", name=f"prod{it}")
                nc.vector.scalar_tensor_tensor(
                    out=prod, in0=r1s[it][:, :], scalar=0.0,
                    in1=dab[:, it * QF:(it + 1) * QF],
                    op0=ALU.bypass, op1=ALU.mult)
                prods.append(prod)
            for it in range(NQ):
                for kk in range(QF // P):
                    col = it * (QF // P) + kk
                    nc.tensor.matmul(risk_pm[:, col:col + 1],
                                     prods[it][:, kk * P:(kk + 1) * P],
                                     ones_col[:, :],
                                     start=True, stop=True,
                                     skip_group_check=True)

            # ---------------- epilogue
            lnr = sb.tile([P, JCH], F32)
            nc.scalar.activation(out=lnr, in_=risk_pm[:, :], func=AF.Ln)
            acc1 = sb.tile([P, 1], F32)
            junk1 = sb.tile([P, JCH], F32)
            nc.vector.scalar_tensor_tensor(out=junk1, in0=lnr, scalar=1.0,
                                           in1=cen_cols, op0=ALU.mult,
                                           op1=ALU.mult, accum_out=acc1)
            sc = sb.tile([P, 1], F32)
            nc.vector.scalar_tensor_tensor(out=sc, in0=acc1, scalar=1.0 / N,
                                           in1=acc2s, op0=ALU.mult,
                                           op1=ALU.subtract)
            fin = pfin.tile([1, 1], F32, tag="fin")
            nc.tensor.matmul(fin[:1, :], sc, ones_col[:, :],
                             start=True, stop=True)
            part = sb.tile([1, 1], F32)
            nc.vector.tensor_copy(part[:1, :], fin[:1, :])
            nc.gpsimd.dma_start(out=out[:, :], in_=part[:1, :])
    return nc


_NC_CACHE = {}


def _get_nc(no_ar=False, early_cc=False):
    key = (no_ar, early_cc)
    if key not in _NC_CACHE:
        nc = build(no_ar=no_ar, early_cc=early_cc)
        legalize_waits(nc)
        _NC_CACHE[key] = nc
    return _NC_CACHE[key]


def _make_in_maps(survtime, censor, hazard_pred):
    s = np.ascontiguousarray(np.asarray(survtime, np.float32).reshape(-1))
    cen = np.ascontiguousarray(np.asarray(censor, np.float32).reshape(-1))
    th = np.ascontiguousarray(np.asarray(hazard_pred, np.float32).reshape(-1))
    assert s.shape == (N,) and cen.shape == (N,) and th.shape == (N,)

    p = np.arange(P, dtype=np.float32)
    ident = np.eye(P, dtype=np.float32)
    iob = np.tile(np.arange(B1, dtype=np.float32)[None, :] * np.float32(B2),
                  (P, 1))
    iot = np.tile(np.arange(B2, dtype=np.float32)[None, :], (P, 1))

    in_maps = []
    for r in range(NCORES):
        sl = slice(r * MY, (r + 1) * MY)
        s_cm = np.ascontiguousarray(s[sl].reshape(JCH, P).T)
        th_cm = np.ascontiguousarray(th[sl].reshape(JCH, P).T)
        cen_cm = np.ascontiguousarray(cen[sl].reshape(JCH, P).T)
        pack = np.concatenate(
            [s_cm, th_cm, cen_cm, p[:, None], (128.0 * p)[:, None],
             np.ones((P, 1), np.float32),
             np.full((P, 1), -0.5, np.float32), iob, iot, ident], axis=1)
        assert pack.shape == (P, PACKW)
        in_maps.append({
            "in_pack": np.ascontiguousarray(pack),
            "in_row": np.ascontiguousarray(s[sl][None, :]),
        })
    return in_maps


def run(survtime, censor, hazard_pred, no_ar=False, early_cc=False, **kw):
    in_maps = _make_in_maps(survtime, censor, hazard_pred)
    res = run_bass_kernel_spmd(_get_nc(no_ar, early_cc), in_maps,
                               list(range(NCORES)), **kw)
    total = np.float64(0.0)
    for r in range(NCORES):
        total += np.float64(np.asarray(res.results[r]["partial"]).reshape(-1)[0])
    return np.asarray(total, dtype=np.float32), res


def kernel(survtime, censor, hazard_pred):
    loss, _ = run(survtime, censor, hazard_pred)
    return loss
